# revision 50
# baseline (speedup 1.0000x reference)
"""BiDAF2 attention kernel for Trainium2, 8-core data parallel over batch.

reference (per batch b):
  w1h[s,l] = h[s,:] @ w1_w[l,:] + w1_b[l]
  w2q[t,l] = q[t,:] @ w2_w[l,:] + w2_b[l]
  a[s,t]   = w1h[s,t] + w2q[t,s] + h[s,:]@q[t,:]
  p        = softmax_t(a);  c[s,:] = p[s,:] @ q
  m[s]     = max_t a[s,t];  p2 = softmax_s(m)
  out      = concat([h, c, h*c, (h*p2)*c], axis=-1)

Strategy per core (2 batches), v7:
  - Logits a[s,t] = hT.uT + w2T.qT + w1_b[t] with uT = qT + w1T built by one
    fused DVE op in TRANSPOSED space (w2_b[s] is row-constant so it drops
    out of softmax_t; the row max is corrected by +w2_b before the p2
    softmax).  26 fp16 matmuls per 128-row s-tile, fp32 PSUM accumulation.
  - Weights are transposed ON THE PE (is_transpose matmul against an
    affine_select identity, PSUM-staged, ACT eviction) — the PE is idle at
    startup and this keeps weight prep off the 8-deep HWDGE transpose ring.
  - q/h arrive as 2-chunk "grains" via Pool-issued SWDGE DMAs (separate
    semaphore ring from the HWDGE transposes), are cast to fp16, and
    f16-transposed per chunk straight into qT/hT.
  - h kept on-chip in fp16 only; out[:,0:D] is fp16(h) cast back (abs err
    ~1e-4 rel).  Epilogue fused: c via ACT copy(scale=1/z), h*c via one DVE
    scalar_tensor_tensor off PSUM, (h*p2)*c via ACT copy(scale=p2); the 4
    sections are assembled contiguously for one 12KB-row store.
  - p2 (softmax over the 1024 row maxes) via a 4KB DRAM-scratch rearrange
    to a single-partition row, softmaxed there, scattered back.
  - Emission is software-pipelined: batch b+1's prep is emitted before batch
    b's phase B; staging rotations are sized so the 8 HWDGE / 8 SWDGE
    completion semaphores recycle among same-stage DMAs.
"""

import os
import sys

for _p in ("/opt/trn_rl_repo", "/root/.axon_site/_ro/trn_rl_repo"):
    if os.path.isdir(_p) and _p not in sys.path:
        sys.path.append(_p)

from contextlib import ExitStack

import numpy as np

import concourse.bass as bass
import concourse.tile as tile
from concourse import bacc, mybir
from concourse.bass_utils import run_bass_kernel_spmd

B, L, D = 16, 1024, 768
NCORES = 8
BL = B // NCORES  # batches per core
P = 128
KD = D // P  # 6 d-chunks
NT = L // P  # 8 t-chunks == 8 s-tiles
F16 = mybir.dt.float16
F32 = mybir.dt.float32
EXP = mybir.ActivationFunctionType.Exp
COPY = mybir.ActivationFunctionType.Copy
AX = mybir.AxisListType.X
MULT = mybir.AluOpType.mult
ADD = mybir.AluOpType.add
SUB = mybir.AluOpType.subtract

HALVES = [(0, 512), (512, 1024)]
DHALVES = [(0, 512), (512, D)]

REPEAT = 1  # benchmarking aid: run the whole body REPEAT times via For_i


def _emit(ctx: ExitStack, tc: tile.TileContext, h, q, w1w, w1b, w2w, w2b, out):
    if REPEAT > 1:
        with tc.For_i(0, REPEAT, 1):
            _emit_once(ctx, tc, h, q, w1w, w1b, w2w, w2b, out)
    else:
        _emit_once(ctx, tc, h, q, w1w, w1b, w2w, w2b, out)


def _emit_once(ctx: ExitStack, tc: tile.TileContext, h, q, w1w, w1b, w2w, w2b, out):
    nc = tc.nc

    singles = ctx.enter_context(tc.tile_pool(name="singles", bufs=1))
    wconst = ctx.enter_context(tc.tile_pool(name="wconst", bufs=1))
    stage32 = ctx.enter_context(tc.tile_pool(name="stage32", bufs=3))
    batchres = ctx.enter_context(tc.tile_pool(name="batchres", bufs=2))
    htile_pool = ctx.enter_context(tc.tile_pool(name="htile", bufs=5))
    pT_pool = ctx.enter_context(tc.tile_pool(name="pTp", bufs=8))
    pstream = ctx.enter_context(tc.tile_pool(name="pstream", bufs=2))
    epil = ctx.enter_context(tc.tile_pool(name="epil", bufs=2))
    smalls = ctx.enter_context(tc.tile_pool(name="smalls", bufs=2))
    dram = ctx.enter_context(tc.tile_pool(name="dram", bufs=2, space="DRAM"))
    psA = ctx.enter_context(tc.tile_pool(name="psA", bufs=2, space="PSUM"))
    psC = ctx.enter_context(tc.tile_pool(name="psC", bufs=2, space="PSUM"))

    # ---- constants ----
    ones1 = singles.tile([1, P], F16)
    nc.vector.memset(ones1, 1.0)
    w1b16 = singles.tile([1, L], F16)
    nc.gpsimd.dma_start(out=w1b16, in_=w1b[None, :])
    w2b_col = singles.tile([P, NT], F32)
    nc.sync.dma_start(out=w2b_col, in_=w2b.rearrange("(c p) -> p c", p=P))
    ident = singles.tile([P, P], F16)
    nc.vector.memset(ident, 1.0)
    nc.gpsimd.affine_select(out=ident, in_=ident, pattern=[[1, P]],
                            compare_op=mybir.AluOpType.is_equal, fill=0.0,
                            base=0, channel_multiplier=-1)

    # ---- once-per-core weights: w1T, w2T (transposed, f16) ----
    # a[s,t] = hT.uT + w2T.qT + w1_b[t]  with uT = qT + w1T (all transposed
    # space).  DMA queues: loads/stores/p2 on SP, all transposes on ACT —
    # each DMA hosted where its producer runs so queue-head waits are
    # pre-satisfied; stage-ordered emission keeps the 8 HWDGE completion
    # sems recycling among same-stage DMAs.
    G = 2
    NG = NT // G
    w1T = wconst.tile([P, KD, L], F16, tag="w1T")
    w2T = wconst.tile([P, KD, L], F16, tag="w2T")
    wgs = []
    for wsrc in (w1w, w2w):
        for g in range(NG):
            rows = slice(g * G * P, (g + 1) * G * P)
            wg = stage32.tile([P, G, D], F32, tag="g32", bufs=7, name="wg")
            nc.gpsimd.dma_start(
                out=wg, in_=wsrc[rows, :].rearrange("(c p) d -> p c d", p=P))
            wgs.append(wg)
    for wi, wT in enumerate((w1T, w2T)):
        for g in range(NG):
            wc = stage32.tile([P, G, D], F16, tag="c16", bufs=4, name="wc")
            if wi == 0:
                nc.gpsimd.tensor_copy(out=wc, in_=wgs[wi * NG + g])
            else:
                nc.vector.tensor_copy(wc, wgs[wi * NG + g])
            for c in range(G):
                cs = slice((g * G + c) * P, (g * G + c + 1) * P)
                # PE-transpose the chunk (PE is idle at startup; keeps the
                # weight prep off the HWDGE transpose ring)
                ps_w32 = psC.tile([P, D], F32, tag="ps_c", name="ps_w32")
                ps_w = ps_w32.bitcast(F16)[:, :D]
                for k in range(KD):
                    nc.tensor.transpose(ps_w[:, k * P:(k + 1) * P],
                                        wc[:, c, k * P:(k + 1) * P], ident)
                nc.scalar.activation(out=wT[:, :, cs], in_=ps_w, func=COPY,
                                     scale=1.0)

    def batch_prep(b, bres):
        q16 = batchres.tile([P, NT, D], F16, tag="q16", name="q16")
        bres["q16"] = q16
        uT = batchres.tile([P, KD, L], F16, tag="uT", name="uT", bufs=1)
        bres["uT"] = uT
        qT = batchres.tile([P, KD, L], F16, tag="qT", name="qT", bufs=1)
        bres["qT"] = qT
        hT = batchres.tile([P, KD, L], F16, tag="hT", name="hT", bufs=1)
        bres["hT"] = hT
        bres["h16s"] = h16s = []

        qgs = []
        for g in range(NG):
            qg = stage32.tile([P, G, D], F32, tag="g32", bufs=7, name="qg")
            nc.gpsimd.dma_start(out=qg, in_=q[b, g * G * P:(g + 1) * G * P, :]
                               .rearrange("(c p) d -> p c d", p=P))
            qgs.append(qg)
        hgs = []
        for g in range(NG):
            hg = stage32.tile([P, G, D], F32, tag="g32", bufs=7, name="hg")
            nc.gpsimd.dma_start(out=hg, in_=h[b, g * G * P:(g + 1) * G * P, :]
                               .rearrange("(c p) d -> p c d", p=P))
            hgs.append(hg)
        for g in range(NG):
            gs = slice(g * G * P, (g + 1) * G * P)
            nc.gpsimd.tensor_copy(out=q16[:, g * G:(g + 1) * G, :], in_=qgs[g])
            for c in range(G):
                cs = slice((g * G + c) * P, (g * G + c + 1) * P)
                nc.scalar.dma_start(out=qT[:, :, cs],
                                    in_=q16[:, g * G + c, :], transpose=True)
            nc.vector.scalar_tensor_tensor(out=uT[:, :, gs], in0=qT[:, :, gs],
                                           scalar=1.0, in1=w1T[:, :, gs],
                                           op0=MULT, op1=ADD)
        for g in range(NG):
            h16g = htile_pool.tile([P, G, D], F16, tag="h16g")
            nc.gpsimd.tensor_copy(out=h16g, in_=hgs[g])
            for c in range(G):
                cs = slice((g * G + c) * P, (g * G + c + 1) * P)
                nc.scalar.dma_start(out=hT[:, :, cs], in_=h16g[:, c, :],
                                    transpose=True)
                h16s.append(h16g[:, c, :])

    def phase_a(b, bres):
        uT, qT, hT = bres["uT"], bres["qT"], bres["hT"]
        m_negcol = smalls.tile([P, NT], F32, tag="m_negcol", name="m_negcol")
        bres["m_negcol"] = m_negcol
        z_col = smalls.tile([P, NT], F32, tag="z_col", name="z_col")
        bres["z_col"] = z_col
        bres["pTs"] = pTs = []
        for i in range(NT):
            s0 = i * P
            # per t-half: bias then h.u / w2.q terms over all k — the t<512
            # half only needs u/q chunks 0-3, so PE starts earlier
            ps_a = psA.tile([P, L], F32)
            for t0, t1 in HALVES:
                nc.tensor.matmul(ps_a[:, t0:t1], ones1, w1b16[:, t0:t1],
                                 start=True, stop=False)
                for k in range(KD):
                    nc.tensor.matmul(ps_a[:, t0:t1], hT[:, k, s0:s0 + P],
                                     uT[:, k, t0:t1], start=False, stop=False)
                    nc.tensor.matmul(ps_a[:, t0:t1], w2T[:, k, s0:s0 + P],
                                     qT[:, k, t0:t1], start=False,
                                     stop=(k == KD - 1))
            negm = m_negcol[:, i:i + 1]
            nc.vector.reduce_max(negm, ps_a, axis=AX, negate=True)
            p16 = pstream.tile([P, L], F16, tag="p16")
            nc.scalar.activation(out=p16, in_=ps_a, func=EXP, bias=negm,
                                 scale=1.0, accum_out=z_col[:, i:i + 1])
            pT = pT_pool.tile([P, NT, P], F16, tag="pT")
            nc.scalar.dma_start(out=pT, in_=p16, transpose=True)
            pTs.append(pT)

    def p2_block(b, bres):
        # p2 = softmax over all 1024 row maxes (depends on phase A only)
        m_true = smalls.tile([P, NT], F32, tag="m_true")
        nc.vector.tensor_sub(m_true, w2b_col, bres["m_negcol"])
        m_dram = dram.tile([L], F32, tag="m_dram")
        nc.sync.dma_start(out=m_dram.rearrange("(c p) -> p c", p=P), in_=m_true)
        m_row = smalls.tile([1, L], F32, tag="row_a", bufs=1)
        nc.sync.dma_start(out=m_row, in_=m_dram[None, :])
        negmm = smalls.tile([1, 1], F32, tag="negmm")
        nc.vector.reduce_max(negmm, m_row, axis=AX, negate=True)
        z2 = smalls.tile([1, 1], F32, tag="z2")
        nc.scalar.activation(out=m_row, in_=m_row, func=EXP, bias=negmm,
                             scale=1.0, accum_out=z2)
        r2 = smalls.tile([1, 1], F32, tag="r2")
        nc.vector.reciprocal(r2, z2)
        nc.vector.tensor_scalar_mul(m_row, in0=m_row, scalar1=r2)
        p2_dram = dram.tile([L], F32, tag="p2_dram")
        nc.sync.dma_start(out=p2_dram[None, :], in_=m_row)
        p2_col = smalls.tile([P, NT], F32, tag="p2_col", name="p2_col")
        bres["p2_col"] = p2_col
        nc.sync.dma_start(out=p2_col, in_=p2_dram.rearrange("(c p) -> p c", p=P))

    def phase_b(b, bres):
        q16, z_col, p2_col = bres["q16"], bres["z_col"], bres["p2_col"]
        r_col = smalls.tile([P, NT], F32, tag="r_col")
        for i in range(NT):
            s0 = i * P
            ps_c = psC.tile([P, D], F32)
            pT = bres["pTs"][i]
            for tcn in range(NT):
                lp = pT[:, tcn, :]
                for d0, d1 in DHALVES:
                    nc.tensor.matmul(ps_c[:, d0:d1], lp, q16[:, tcn, d0:d1],
                                     start=(tcn == 0), stop=(tcn == NT - 1))
            r_i = r_col[:, i:i + 1]
            nc.vector.reciprocal(r_i, z_col[:, i:i + 1])
            osec = epil.tile([P, 4, D], F32, tag="osec")
            nc.gpsimd.tensor_copy(out=osec[:, 0, :], in_=bres["h16s"][i])
            nc.scalar.activation(out=osec[:, 1, :], in_=ps_c, func=COPY,
                                 scale=r_i)
            nc.vector.scalar_tensor_tensor(out=osec[:, 2, :], in0=ps_c,
                                           scalar=r_i, in1=osec[:, 0, :],
                                           op0=MULT, op1=MULT)
            nc.scalar.activation(out=osec[:, 3, :], in_=osec[:, 2, :],
                                 func=COPY, scale=p2_col[:, i:i + 1])
            nc.sync.dma_start(out=out[b, s0:s0 + P, :], in_=osec)

    # software-pipelined emission: batch b+1's prep is emitted before batch
    # b's phase B so the PE never waits on operand prep at batch boundaries
    bres = [{} for _ in range(BL)]
    batch_prep(0, bres[0])
    for b in range(BL):
        phase_a(b, bres[b])
        if b + 1 < BL:
            batch_prep(b + 1, bres[b + 1])
        p2_block(b, bres[b])
        phase_b(b, bres[b])


def build():
    nc = bacc.Bacc()
    h = nc.dram_tensor("h", [BL, L, D], F32, kind="ExternalInput")
    q = nc.dram_tensor("q", [BL, L, D], F32, kind="ExternalInput")
    w1w = nc.dram_tensor("w1_w", [L, D], F32, kind="ExternalInput")
    w1b = nc.dram_tensor("w1_b", [L], F32, kind="ExternalInput")
    w2w = nc.dram_tensor("w2_w", [L, D], F32, kind="ExternalInput")
    w2b = nc.dram_tensor("w2_b", [L], F32, kind="ExternalInput")
    out = nc.dram_tensor("out", [BL, L, 4 * D], F32, kind="ExternalOutput")
    with tile.TileContext(nc) as tc, ExitStack() as ctx:
        _emit(ctx, tc, h[:], q[:], w1w[:], w1b[:], w2w[:], w2b[:], out[:])
    nc.compile()
    return nc


def _in_maps(inputs):
    arr = {k: np.ascontiguousarray(np.asarray(v, np.float32))
           for k, v in inputs.items()}
    maps = []
    for c in range(NCORES):
        sl = slice(c * BL, (c + 1) * BL)
        maps.append({
            "h": arr["h"][sl], "q": arr["q"][sl],
            "w1_w": arr["w1_w"], "w1_b": arr["w1_b"],
            "w2_w": arr["w2_w"], "w2_b": arr["w2_b"],
        })
    return maps


def kernel(**inputs):
    nc = build()
    res = run_bass_kernel_spmd(nc, _in_maps(inputs), core_ids=list(range(NCORES)))
    return np.concatenate([r["out"] for r in res.results], axis=0)


def run_profiled(inputs, **kwargs):
    nc = build()
    res = run_bass_kernel_spmd(
        nc, _in_maps(inputs), core_ids=list(range(NCORES)), trace=True, **kwargs
    )
    out = np.concatenate([r["out"] for r in res.results], axis=0)
    return out, res
